# revision 1
# baseline (speedup 1.0000x reference)
"""Contourlet transform kernel for 8 Trainium2 NeuronCores.

Input x: [16, 32, 512, 512] f32 -> output [16, 32, 9681] f32.

Strategy: 512 independent (b,c) planes, 64 per core, 4 batches of 16.
Each plane is split into 8 row-blocks of 64 rows; SBUF partition =
(plane_in_batch, block), plane data lives in the free dimension, so both
row and column 2-tap DWT passes are strided free-dim scalar_tensor_tensor
ops on the vector engine (no transposes anywhere).

Every 2-tap pass computes (a * (f1/f0) + b), i.e. the true output divided
by f0.  The dropped factors accumulate multiplicatively down the cascade;
kept subbands are fixed up by a single scaled-copy on the scalar engine
into the output staging tile.  Once the LL chain reaches 16x16 the block
layout runs out of rows, so planes are repacked to one-plane-per-partition
([64, 256]) and the remaining levels run there; everything below 2x2 is a
rank-1 linear map of the 1x1 LL value, applied as one tensor_scalar op
with 126 host-precomputed constants.

The device writes a packed layout (OUT_BLK [512,1176] + OUT_TAIL [64,273]
per core); the host gather applies a fixed permutation per plane.
"""

import numpy as np

INV_SQRT2 = 0.7071067811865476

# ---- fixed geometry -------------------------------------------------------
NPLANES = 512          # 16*32
NCORES = 8
PPC = 64               # planes per core
NBATCH = 4             # batches per core
BPL = 16               # planes per batch
NBLK = 8               # row-blocks per plane
ROWS_PER_BLK = 64      # 512 / NBLK
NSC = 8                # L1 sub-chunks per batch
SC_ROWS = 8            # rows per sub-chunk per block

# per-partition offsets of the scale0 l=0,1,2 subband regions in OUT_BLK
LOFF = [0, 896, 1120]          # 7*128, 7*32, 7*8
BLK_FLOATS = 1176              # per-partition OUT_BLK floats
# OUT_TAIL per-plane offsets
TOFF = {3: 0, 4: 112, 5: 140}  # 7*16, 7*4, 7*1
TCONST = 147                   # 126 map outputs
TAIL_FLOATS = 273


# ---- backends -------------------------------------------------------------
class NpTile:
    """numpy [P, F] tile with bass-AP-like 3-d reshaping."""

    def __init__(self, arr):
        self.arr = arr

    def __getitem__(self, key):
        return self.arr[key]

    def __setitem__(self, key, val):
        self.arr[key] = val


class NumpyBE:
    """Numpy mirror of the device op plan (1 core)."""

    def __init__(self, xs, h, g, tmap):
        # xs: [64, 512, 512] planes for this core
        self.xs, self.h, self.g = xs, h, g
        self.tmap = tmap  # [126]
        self.out_blk = np.zeros((NBATCH * 128, BLK_FLOATS), np.float32)
        self.out_tail = np.zeros((PPC, TAIL_FLOATS), np.float32)

    def alloc(self, name, shape):
        return NpTile(np.zeros(shape, np.float32))

    @staticmethod
    def r3(tile, cols, sub=None):
        """view tile (or its free-slice sub=(start,len)) as [P, rows, cols]"""
        arr = tile.arr if isinstance(tile, NpTile) else tile
        if sub is not None:
            arr = arr[:, sub[0]:sub[0] + sub[1]]
        P, F = arr.shape
        return arr.reshape(P, F // cols, cols)

    def stt(self, out, a, s, b):
        out[...] = a * np.float32(s) + b

    def scale_copy(self, out, inp, s):
        out[...] = inp * np.float32(s)

    def ts_mul(self, out, a, col):
        out[...] = a * col  # col: [P,1]

    def load_x_chunk(self, t, sc, dst, nsc=NSC):
        # dst [128, sc_rows*512]: partition (pl, blk) <- plane 16t+pl,
        # rows blk*64 + sc*sc_rows .. +sc_rows, all 512 cols
        sc_rows = ROWS_PER_BLK // nsc
        x = self.xs[t * BPL:(t + 1) * BPL]  # [16, 512, 512]
        v = x.reshape(BPL, NBLK, nsc, sc_rows, 512)[:, :, sc]
        dst.arr[...] = v.reshape(128, sc_rows * 512)

    def repack_tail(self, t, ll, tail):
        # ll [128, 32] -> tail[16t:16t+16, :]: plane-major 16x16
        tail.arr[t * BPL:(t + 1) * BPL] = ll.arr.reshape(BPL, NBLK * 32)

    def store_outb(self, t, outb):
        self.out_blk[t * 128:(t + 1) * 128] = outb.arr

    def store_outt(self, outt):
        self.out_tail[...] = outt.arr

    def load_tmap(self, dst):
        dst.arr[...] = np.broadcast_to(self.tmap, (PPC, 126))


# ---- shared op plan -------------------------------------------------------
def emit_direction(be, LL, R, S, l, s, dst_tile, dst_off, P, h, g):
    """One directional decomposition: dwt2(LL, h[l]) -> LL,LH,HL,HH then
    g-decompositions keeping (A1,H1,V1,A2,H2,H3,D3) into dst at dst_off.
    LL: [P, R*S] tile viewed as (R rows, S cols) per partition.
    Returns (LL_next tile [P, (R/2)*(S/2)], new scale)."""
    f0, f1 = float(h[l, 0]), float(h[l, 1])
    g0, g1 = float(g[l, 0]), float(g[l, 1])
    rh, rg = f1 / f0, g1 / g0
    S2, R2 = S // 2, R // 2
    m = S // 4
    R4 = R // 4 if R >= 4 else 1   # rows/partition of kept bands
    L3 = be.r3(LL, S)

    CL = be.alloc("cl", [P, R * S2])
    CH = be.alloc("ch", [P, R * S2])
    be.stt(be.r3(CL, S2), L3[:, :, 0::2], rh, L3[:, :, 1::2])
    be.stt(be.r3(CH, S2), L3[:, :, 1::2], -rh, L3[:, :, 0::2])

    C3L, C3H = be.r3(CL, S2), be.r3(CH, S2)
    LLn = be.alloc("lln", [P, R2 * S2])
    LH = be.alloc("lh", [P, R2 * S2])
    HL = be.alloc("hl", [P, R2 * S2])
    HH = be.alloc("hh", [P, R2 * S2])
    be.stt(be.r3(LLn, S2), C3L[:, 0::2, :], rh, C3L[:, 1::2, :])
    be.stt(be.r3(LH, S2), C3L[:, 1::2, :], -rh, C3L[:, 0::2, :])
    be.stt(be.r3(HL, S2), C3H[:, 0::2, :], rh, C3H[:, 1::2, :])
    be.stt(be.r3(HH, S2), C3H[:, 1::2, :], -rh, C3H[:, 0::2, :])

    # g-stage on bands of size S2 (R2 rows/partition); kept bands mxm, R4 rows
    q = R4 * m                       # floats per kept band per partition
    SCR = be.alloc("scr", [P, 7 * q])

    def scr(i):
        return be.r3(SCR, m, sub=(i * q, q))

    GL = be.alloc("gl", [P, R2 * m])
    GH = be.alloc("gh", [P, R2 * m])

    # LH -> A1 (row-lo col-lo), H1 (row-hi col-lo), V1 (row-lo col-hi)
    B3 = be.r3(LH, S2)
    be.stt(be.r3(GL, m), B3[:, :, 0::2], rg, B3[:, :, 1::2])
    be.stt(be.r3(GH, m), B3[:, :, 1::2], -rg, B3[:, :, 0::2])
    G3L, G3H = be.r3(GL, m), be.r3(GH, m)
    be.stt(scr(0), G3L[:, 0::2, :], rg, G3L[:, 1::2, :])
    be.stt(scr(1), G3L[:, 1::2, :], -rg, G3L[:, 0::2, :])
    be.stt(scr(2), G3H[:, 0::2, :], rg, G3H[:, 1::2, :])

    # HL -> A2 (row-lo col-lo), H2 (row-hi col-lo): col-lo branch only
    GL2 = be.alloc("gl2", [P, R2 * m])
    B3 = be.r3(HL, S2)
    be.stt(be.r3(GL2, m), B3[:, :, 0::2], rg, B3[:, :, 1::2])
    G3L = be.r3(GL2, m)
    be.stt(scr(3), G3L[:, 0::2, :], rg, G3L[:, 1::2, :])
    be.stt(scr(4), G3L[:, 1::2, :], -rg, G3L[:, 0::2, :])

    # HH -> H3 (row-hi col-lo), D3 (row-hi col-hi)
    GL3 = be.alloc("gl3", [P, R2 * m])
    GH3 = be.alloc("gh3", [P, R2 * m])
    B3 = be.r3(HH, S2)
    be.stt(be.r3(GL3, m), B3[:, :, 0::2], rg, B3[:, :, 1::2])
    be.stt(be.r3(GH3, m), B3[:, :, 1::2], -rg, B3[:, :, 0::2])
    G3L, G3H = be.r3(GL3, m), be.r3(GH3, m)
    be.stt(scr(5), G3L[:, 1::2, :], -rg, G3L[:, 0::2, :])
    be.stt(scr(6), G3H[:, 1::2, :], -rg, G3H[:, 0::2, :])

    s_band = s * (f0 * f0) * (g0 * g0)
    be.scale_copy(dst_tile[:, dst_off:dst_off + 7 * q], SCR[:, :], s_band)
    return LLn, s * f0 * f0


def emit_core(be, h, g):
    """Full per-core program."""
    c = INV_SQRT2
    nsc = getattr(be, 'opts', {}).get('nsc', NSC) if hasattr(be, 'opts') else NSC
    sc_rows = ROWS_PER_BLK // nsc
    TAILLL = be.alloc("tailll", [PPC, 256])
    TMAP = be.alloc("tmap", [PPC, 126])
    be.load_tmap(TMAP)

    s_tail = None
    for t in range(NBATCH):
        OUTB = be.alloc("outb", [128, BLK_FLOATS])
        LL2 = be.alloc("ll2", [128, 16 * 128])
        for sc in range(nsc):
            XT = be.alloc("xt", [128, sc_rows * 512])
            be.load_x_chunk(t, sc, XT, nsc)
            X3 = be.r3(XT, 512)
            U = be.alloc("u", [128, (sc_rows // 2) * 512])
            be.stt(be.r3(U, 512), X3[:, 0::2, :], 1.0, X3[:, 1::2, :])
            U3 = be.r3(U, 512)
            orows = sc_rows // 2   # L1 rows from this chunk (even count)
            LL1C = be.alloc("ll1c", [128, orows * 256])
            be.stt(be.r3(LL1C, 256), U3[:, :, 0::2], 1.0, U3[:, :, 1::2])
            # L2 Haar on this chunk's L1 rows (pairs stay inside the chunk)
            L13 = be.r3(LL1C, 256)
            U2C = be.alloc("u2c", [128, (orows // 2) * 256])
            be.stt(be.r3(U2C, 256), L13[:, 0::2, :], 1.0, L13[:, 1::2, :])
            U23 = be.r3(U2C, 256)
            ll2_rows = orows // 2
            ll2_slice = be.r3(LL2, 128)[:, sc * ll2_rows:(sc + 1) * ll2_rows, :]
            be.stt(ll2_slice, U23[:, :, 0::2], 1.0, U23[:, :, 1::2])

        LL, s, R, S = LL2, c ** 4, 16, 128
        for l in range(3):
            LL, s = emit_direction(be, LL, R, S, l, s, OUTB, LOFF[l], 128, h, g)
            R, S = R // 2, S // 2
        # LL now [128, 2*16] = 16x16 plane spread over 8 blocks
        be.repack_tail(t, LL, TAILLL)
        be.store_outb(t, OUTB)
        s_tail = s

    # ---- tail: plane-major [64 planes, ...] --------------------------------
    OUTT = be.alloc("outt", [PPC, TAIL_FLOATS])
    LL, s, R, S = TAILLL, s_tail, 16, 16
    for l in (3, 4, 5):
        LL, s = emit_direction(be, LL, R, S, l, s, OUTT, TOFF[l], PPC, h, g)
        R, S = R // 2, S // 2
    # LL: [64, 4] = 2x2.  scale1 Haar -> 1x1
    CT = be.alloc("ct", [PPC, 2])
    L3 = be.r3(LL, 2)
    be.stt(be.r3(CT, 1), L3[:, :, 0:1], 1.0, L3[:, :, 1:2])
    LL11 = be.alloc("ll11", [PPC, 1])
    be.stt(LL11[:, 0:1], CT[:, 0:1], 1.0, CT[:, 1:2])
    # scales 1-3 for all 6 directions: rank-1 map of LL11 (consts incl. s)
    be.ts_mul(OUTT[:, TCONST:TCONST + 126], TMAP[:, :], LL11[:, 0:1])
    be.store_outt(OUTT)
    return s * c * c  # scale of LL11 (true = s11 * raw); informational


# ---- host-side constants --------------------------------------------------
def _dwt2_np(x, f0, f1):
    def dwt_last(x):
        n = x.shape[-1]
        m = (n + 1) // 2
        xe = np.pad(x, [(0, 0)] * (x.ndim - 1) + [(1, 1)], mode='edge')
        a = xe[..., 1:2 * m + 1:2]
        b = xe[..., 2:2 * m + 2:2]
        return f1 * a + f0 * b, f0 * a - f1 * b

    lo, hi = dwt_last(x)
    lo, hi = np.swapaxes(lo, -1, -2), np.swapaxes(hi, -1, -2)
    ll, lh = dwt_last(lo)
    hl, hh = dwt_last(hi)
    sw = lambda t: np.swapaxes(t, -1, -2)
    return sw(ll), sw(lh), sw(hl), sw(hh)


def build_tail_consts(h, g, s11):
    """126 constants: scales 1-3 outputs as multiples of the raw 1x1 LL."""
    c = INV_SQRT2
    h = np.asarray(h, np.float64)
    g = np.asarray(g, np.float64)
    LL = np.ones((1, 1))
    vals = []
    for k in range(1, 4):
        if k > 1:
            LL, _, _, _ = _dwt2_np(LL, c, c)
        for l in range(6):
            LL, LH, HL, HH = _dwt2_np(LL, h[l, 0], h[l, 1])
            A1, H1, V1, _ = _dwt2_np(LH, g[l, 0], g[l, 1])
            A2, H2, _, _ = _dwt2_np(HL, g[l, 0], g[l, 1])
            _, H3, _, D3 = _dwt2_np(HH, g[l, 0], g[l, 1])
            for sb in (A1, H1, V1, A2, H2, H3, D3):
                vals.append(float(sb[0, 0]))
    return (np.asarray(vals, np.float64) * s11).astype(np.float32)


def tail_scale(h, g):
    """scale s11 of the raw 1x1 LL value (true = s11 * raw)."""
    c = INV_SQRT2
    s = c ** 4  # L1 + L2 Haar drops
    for l in range(6):
        s *= float(h[l, 0]) ** 2
    return s * c * c  # scale1 Haar drops


def build_perm():
    """perm[ref_pos] = index into per-plane packed vector
    v = concat(OUT_BLK rows for blocks 0..7 (8*1176), OUT_TAIL row (273))."""
    perm = np.empty(9681, np.int64)
    off = 0
    for l, m in enumerate((32, 16, 8)):
        rpb = m // NBLK
        loff = LOFF[l]
        for sb in range(7):
            for row in range(m):
                blk, rl = divmod(row, rpb)
                base = blk * BLK_FLOATS + loff + sb * rpb * m + rl * m
                perm[off + sb * m * m + row * m:off + sb * m * m + (row + 1) * m] = \
                    np.arange(base, base + m)
        off += 7 * m * m
    tail_base = NBLK * BLK_FLOATS
    for l, m in ((3, 4), (4, 2), (5, 1)):
        n = 7 * m * m
        perm[off:off + n] = tail_base + TOFF[l] + np.arange(n)
        off += n
    perm[off:off + 126] = tail_base + TCONST + np.arange(126)
    assert off + 126 == 9681
    return perm


def gather_host(out_blk, out_tail, perm):
    """[512,1176],[64,273] per core -> [64, 9681] in reference order."""
    v = np.concatenate(
        [out_blk.reshape(NBATCH, BPL, NBLK * BLK_FLOATS).reshape(PPC, -1),
         out_tail], axis=1)
    return v[:, perm]


# ---- device backend -------------------------------------------------------
class BassBE:
    """Emits the op plan as a Tile program."""

    def __init__(self, tc, pools, xs_ap, tmap_ap, outblk_ap, outtail_ap,
                 dram_bounce, opts=None):
        self.opts = opts or {}
        self.tc = tc
        self.nc = tc.nc
        self.pools = pools
        self.xs = xs_ap          # [64, 512, 512] dram
        self.tmap_dram = tmap_ap  # [64, 126] dram
        self.outblk = outblk_ap  # [512, 1176] dram
        self.outtail = outtail_ap  # [64, 273] dram
        self.bounce = dram_bounce  # [128, 32] dram scratch

    def alloc(self, name, shape):
        from concourse import mybir
        if name in ('tailll', 'tmap'):
            pool = self.pools['persist']
        elif name == 'll2':
            pool = self.pools['big']
        elif name == 'xt':
            pool = self.pools['xt']
        else:
            pool = self.pools['work']
        return pool.tile(list(shape), mybir.dt.float32, tag=name, name=name)

    @staticmethod
    def r3(tile, cols, sub=None):
        ap = tile[:, :] if not hasattr(tile, 'ap') else tile[:, :]
        if sub is not None:
            ap = ap[:, sub[0]:sub[0] + sub[1]]
        P, F = ap.shape
        return ap.rearrange("p (r c) -> p r c", c=cols)

    def stt(self, out, a, s, b):
        from concourse import mybir
        self.nc.vector.scalar_tensor_tensor(
            out=out, in0=a, scalar=float(s), in1=b,
            op0=mybir.AluOpType.mult, op1=mybir.AluOpType.add)

    def scale_copy(self, out, inp, s):
        if self.opts.get('comp_engine', 'scalar') == 'vector':
            from concourse import mybir
            self.nc.vector.tensor_scalar(
                out=out, in0=inp, scalar1=float(s), scalar2=None,
                op0=mybir.AluOpType.mult)
        else:
            self.nc.scalar.mul(out, inp, float(s))

    def ts_mul(self, out, a, col):
        from concourse import mybir
        self.nc.vector.tensor_scalar(
            out=out, in0=a, scalar1=col, scalar2=None,
            op0=mybir.AluOpType.mult)

    def load_x_chunk(self, t, sc, dst, nsc=NSC):
        v = self.xs.rearrange("pl (blk s r) c -> pl blk s r c", blk=NBLK, s=nsc)
        v = v[t * BPL:(t + 1) * BPL, :, sc]
        v = v.rearrange("pl blk r c -> (pl blk) (r c)")
        self.nc.sync.dma_start(out=dst[:, :], in_=v)

    def repack_tail(self, t, ll, tail):
        # [128, 32] sbuf -> dram bounce -> tail[16t:16t+16, :] ([16, 256])
        self.nc.sync.dma_start(out=self.bounce[:, :], in_=ll[:, :])
        src = self.bounce.rearrange("(pl b) j -> pl (b j)", b=NBLK)
        self.nc.sync.dma_start(out=tail[t * BPL:(t + 1) * BPL, :], in_=src)

    def store_outb(self, t, outb):
        self.nc.sync.dma_start(
            out=self.outblk[t * 128:(t + 1) * 128, :], in_=outb[:, :])

    def store_outt(self, outt):
        self.nc.sync.dma_start(out=self.outtail[:, :], in_=outt[:, :])

    def load_tmap(self, dst):
        self.nc.sync.dma_start(out=dst[:, :], in_=self.tmap_dram[:, :])


def build_program(h, g, opts=None):
    """Builds the single-core SPMD Tile program. Returns compiled nc."""
    from contextlib import ExitStack
    import concourse.bacc as bacc
    import concourse.tile as tile
    from concourse import mybir

    opts = opts or {}
    nc = bacc.Bacc("TRN2", target_bir_lowering=False, debug=False,
                   num_devices=NCORES)
    xs = nc.dram_tensor("xs", [PPC, 512, 512], mybir.dt.float32,
                        kind="ExternalInput").ap()
    tmap = nc.dram_tensor("tmap", [PPC, 126], mybir.dt.float32,
                          kind="ExternalInput").ap()
    outblk = nc.dram_tensor("out_blk", [NBATCH * 128, BLK_FLOATS],
                            mybir.dt.float32, kind="ExternalOutput").ap()
    outtail = nc.dram_tensor("out_tail", [PPC, TAIL_FLOATS],
                             mybir.dt.float32, kind="ExternalOutput").ap()
    bounce = nc.dram_tensor("bounce", [128, 32], mybir.dt.float32).ap()

    with ExitStack() as ctx:
        tc = ctx.enter_context(tile.TileContext(nc, trace_sim=False))
        pools = {
            'work': ctx.enter_context(
                tc.tile_pool(name="work", bufs=opts.get('work_bufs', 2))),
            'xt': ctx.enter_context(
                tc.tile_pool(name="xt", bufs=opts.get('xt_bufs', 2))),
            'big': ctx.enter_context(
                tc.tile_pool(name="big", bufs=opts.get('big_bufs', 1))),
            'persist': ctx.enter_context(tc.tile_pool(name="persist", bufs=1)),
        }
        be = BassBE(tc, pools, xs, tmap, outblk, outtail, bounce, opts)
        for _ in range(opts.get('repeat', 1)):
            emit_core(be, h, g)
    nc.compile()
    return nc


# ---- public entry ---------------------------------------------------------
_CACHE = {}


def kernel(x, h, g):
    x = np.ascontiguousarray(np.asarray(x), dtype=np.float32)
    h = np.asarray(h, np.float32)
    g = np.asarray(g, np.float32)
    B, C = x.shape[0], x.shape[1]

    from concourse.bass_utils import run_bass_kernel_spmd

    key = (h.tobytes(), g.tobytes())
    if key not in _CACHE:
        nc = build_program(h, g, {'nsc': 8, 'xt_bufs': 4})
        tmap_row = build_tail_consts(h, g, tail_scale(h, g))
        tmap = np.ascontiguousarray(
            np.broadcast_to(tmap_row, (PPC, 126)), dtype=np.float32)
        perm = build_perm()
        _CACHE[key] = (nc, tmap, perm)
    nc, tmap, perm = _CACHE[key]

    planes = x.reshape(NPLANES, 512, 512)
    in_maps = [{"xs": planes[k * PPC:(k + 1) * PPC], "tmap": tmap}
               for k in range(NCORES)]
    res = run_bass_kernel_spmd(nc, in_maps, list(range(NCORES)))
    global LAST_EXEC_NS
    LAST_EXEC_NS = getattr(res, 'exec_time_ns', None)
    out = np.empty((NPLANES, 9681), np.float32)
    for k in range(NCORES):
        out[k * PPC:(k + 1) * PPC] = gather_host(
            res.results[k]["out_blk"], res.results[k]["out_tail"], perm)
    return out.reshape(B, C, 9681)



# revision 2
# speedup vs baseline: 77365.1266x; 77365.1266x over previous
"""Contourlet transform kernel for 8 Trainium2 NeuronCores (v2).

Input x: [16, 32, 512, 512] f32 -> output [16, 32, 9681] f32.

v2 design (vs v1): the whole on-chip pipeline runs in bf16.
- Input is cast f32->bf16 during the DMA load (SWDGE), halving SBUF
  write traffic and enabling DVE 2x packed modes.
- The two Haar levels (512->256->128) are plain tensor_add ops (the
  1/sqrt2 taps are dropped and fixed up at the end), streamed over
  8 chunks of 8 MiB; row-pair adds hit the bf16 2x mode.
- The three scale-1 direction chains (l=0,1,2) process ALL FOUR
  batches in single ops: the LL tile keeps the batch index as a middle
  free dim (r, b, c), so row ops fold (b c) and col ops fold (r b) and
  every op stays a 3-D access pattern. 4x fewer instructions.
- Kept subbands are written in device-native order; the host gather
  applies a per-batch permutation (build_perm2).
- The 16x16 tail is repacked plane-major with one SBUF->SBUF DMA and
  processed as in v1; scales 2-4 are a rank-1 map with host consts.

Every 2-tap op drops constant factors (Haar: both taps, so plain add;
directional: divide by f0, so (a*(f1/f0) + b)); kept subbands get one
scaled-copy fixup on the scalar engine.
"""

import numpy as np

INV_SQRT2 = 0.7071067811865476

# ---- fixed geometry -------------------------------------------------------
NPLANES = 512          # 16*32
NCORES = 8
PPC = 64               # planes per core
NBATCH = 4             # batches per core
BPL = 16               # planes per batch
NBLK = 8               # row-blocks per plane
NCHUNK = 8             # input chunks per core (one per (batch, half))

# OUT_BLK regions: one contiguous region per (batch-set, l), in emission
# order; region holds [sb7, r(R4), b_local(nb), m] contiguous.
DEF_SETS = ((0, 2), (2, 1), (3, 1))
BLK_FLOATS = 4704               # per-partition OUT_BLK floats (= 4*1176)
TOFF = {3: 0, 4: 112, 5: 140}   # 7*16, 7*4, 7*1 (per plane)
TCONST = 147
TAIL_FLOATS = 273


def region_offsets(sets=DEF_SETS):
    """{(start, l): (offset, length)} for the per-(set, l) OUT_BLK regions."""
    offs = {}
    off = 0
    for start, nb in sets:
        for l, mm in enumerate((32, 16, 8)):
            R4 = mm // NBLK
            ln = 7 * R4 * nb * mm
            offs[(start, l)] = (off, ln)
            off += ln
    assert off == BLK_FLOATS
    return offs


# ---- numpy mirror of the device plan (1 core) -----------------------------
class NpTile:
    def __init__(self, arr):
        self.arr = arr

    def __getitem__(self, key):
        return self.arr[key]

    def __setitem__(self, key, val):
        self.arr[key] = val


class NumpyBE:
    def __init__(self, xs, h, g, tmap, mtail=None):
        self.xs, self.h, self.g = xs, h, g
        self.tmap = tmap
        self.mtail = mtail
        self.out_blk = np.zeros((128, BLK_FLOATS), np.float32)
        self.out_tail = np.zeros((PPC, TAIL_FLOATS), np.float32)

    def prep_tail_mm(self):
        pass

    def emit_tail_mm(self, tail):
        self.out_tail[...] = (tail.arr @ self.mtail).astype(np.float32)

    def alloc(self, name, shape, dtype='bf16'):
        return NpTile(np.zeros(shape, np.float32))

    @staticmethod
    def view(tile, dims, sub=None):
        """view tile free dim as [P, *dims]; sub=(start,len) slices first."""
        arr = tile.arr if isinstance(tile, NpTile) else tile
        if sub is not None:
            arr = arr[:, sub[0]:sub[0] + sub[1]]
        P = arr.shape[0]
        return arr.reshape(P, *dims)

    def add(self, out, a, b):
        out[...] = a + b

    def stt(self, out, a, s, b):
        out[...] = a * np.float32(s) + b

    def scale_copy(self, out, inp, s):
        out[...] = inp * np.float32(s)

    def ts_mul(self, out, a, col):
        out[...] = a * col

    def load_chunk(self, t, q, dst, nq=4):
        # dst [128, rw*512]: partition (pl, blk) <- plane 16t+pl,
        # rows blk*64 + q*rw .. +rw, all cols
        rw = 64 // nq
        x = self.xs[t * BPL:(t + 1) * BPL]
        v = x.reshape(BPL, NBLK, nq, rw, 512)[:, :, q]
        dst.arr[...] = v.reshape(128, rw * 512)

    def repack_set(self, ll3, start, nb, tail):
        # ll3 [128=(pl,blk), (r2, b_nb, c16)] -> tail rows [(b,pl), (blk,r,c)]
        v = ll3.arr.reshape(BPL, NBLK, 2, nb, 16)
        for i in range(nb):
            tail.arr[(start + i) * BPL:(start + i + 1) * BPL] = \
                v[:, :, :, i, :].reshape(BPL, NBLK * 32)

    def store_outb_set(self, outb, base, ln):
        self.out_blk[:, base:base + ln] = outb.arr

    def load_tmap(self, dst):
        dst.arr[...] = np.broadcast_to(self.tmap, (PPC, 126))


# ---- shared op plan -------------------------------------------------------
def emit_direction(be, LL, R, S, NB, l, s, dst_tile, dst_off, P, h, g,
                   rows=None, tiles=None):
    """One directional decomposition, NB batches wide.
    LL: [P, R*NB*S] viewed (R rows, NB batch, S cols) per partition.
    Row ops fold (b c); col ops fold (r b); all APs stay 3-D.
    Kept bands land contiguously at dst_off as [sb7, R4, NB, m].
    rows=(a, b): process only input rows a..b (both multiples of 4) --
    caller passes a shared `tiles` dict and calls once per row range; the
    band fixup fires on the final range.
    Returns (LL_next [P, (R/2)*NB*(S/2)], new scale)."""
    f0, f1 = float(h[l, 0]), float(h[l, 1])
    g0, g1 = float(g[l, 0]), float(g[l, 1])
    rh, rg = f1 / f0, g1 / g0
    S2, R2 = S // 2, R // 2
    m = S // 4
    R4 = R // 4 if R >= 4 else 1
    a, b = rows if rows is not None else (0, R)

    def cview(tile, rows_, cols, lo, hi, sub=None):
        return be.view(tile, (rows_ * NB, cols), sub)[:, lo * NB:hi * NB, :]

    def rview(tile, rows_, cols, lo, hi, sub=None):
        return be.view(tile, (rows_, NB * cols), sub)[:, lo:hi, :]

    def alloc(name, shape):
        if tiles is None:
            return be.alloc(name, shape)
        if name not in tiles:
            tiles[name] = be.alloc(name, shape)
        return tiles[name]

    # h-stage cols
    L3 = cview(LL, R, S, a, b)
    CL = alloc("cl", [P, R * NB * S2])
    CH = alloc("ch", [P, R * NB * S2])
    be.stt(cview(CL, R, S2, a, b), L3[:, :, 0::2], rh, L3[:, :, 1::2])
    be.stt(cview(CH, R, S2, a, b), L3[:, :, 1::2], -rh, L3[:, :, 0::2])

    # h-stage rows
    C3L, C3H = rview(CL, R, S2, a, b), rview(CH, R, S2, a, b)
    LLn = alloc("lln", [P, R2 * NB * S2])
    LH = alloc("lh", [P, R2 * NB * S2])
    HL = alloc("hl", [P, R2 * NB * S2])
    HH = alloc("hh", [P, R2 * NB * S2])
    a2, b2 = a // 2, b // 2
    be.stt(rview(LLn, R2, S2, a2, b2), C3L[:, 0::2, :], rh, C3L[:, 1::2, :])
    be.stt(rview(LH, R2, S2, a2, b2), C3L[:, 1::2, :], -rh, C3L[:, 0::2, :])
    be.stt(rview(HL, R2, S2, a2, b2), C3H[:, 0::2, :], rh, C3H[:, 1::2, :])
    be.stt(rview(HH, R2, S2, a2, b2), C3H[:, 1::2, :], -rh, C3H[:, 0::2, :])

    # g-stage; kept bands are [R4, NB, m] per partition
    q = R4 * NB * m
    SCR = alloc("scr", [P, 7 * q])
    a4, b4 = a // 4, (b + 3) // 4

    def scr(i):
        return rview(SCR, R4, m, a4, b4, sub=(i * q, q))

    GL = alloc("gl", [P, R2 * NB * m])
    GH = alloc("gh", [P, R2 * NB * m])

    # LH -> A1, H1, V1
    B3 = cview(LH, R2, S2, a2, b2)
    be.stt(cview(GL, R2, m, a2, b2), B3[:, :, 0::2], rg, B3[:, :, 1::2])
    be.stt(cview(GH, R2, m, a2, b2), B3[:, :, 1::2], -rg, B3[:, :, 0::2])
    G3L, G3H = rview(GL, R2, m, a2, b2), rview(GH, R2, m, a2, b2)
    be.stt(scr(0), G3L[:, 0::2, :], rg, G3L[:, 1::2, :])
    be.stt(scr(1), G3L[:, 1::2, :], -rg, G3L[:, 0::2, :])
    be.stt(scr(2), G3H[:, 0::2, :], rg, G3H[:, 1::2, :])

    # HL -> A2, H2 (col-lo branch only)
    GL2 = alloc("gl2", [P, R2 * NB * m])
    B3 = cview(HL, R2, S2, a2, b2)
    be.stt(cview(GL2, R2, m, a2, b2), B3[:, :, 0::2], rg, B3[:, :, 1::2])
    G3L = rview(GL2, R2, m, a2, b2)
    be.stt(scr(3), G3L[:, 0::2, :], rg, G3L[:, 1::2, :])
    be.stt(scr(4), G3L[:, 1::2, :], -rg, G3L[:, 0::2, :])

    # HH -> H3, D3
    GL3 = alloc("gl3", [P, R2 * NB * m])
    GH3 = alloc("gh3", [P, R2 * NB * m])
    B3 = cview(HH, R2, S2, a2, b2)
    be.stt(cview(GL3, R2, m, a2, b2), B3[:, :, 0::2], rg, B3[:, :, 1::2])
    be.stt(cview(GH3, R2, m, a2, b2), B3[:, :, 1::2], -rg, B3[:, :, 0::2])
    G3L, G3H = rview(GL3, R2, m, a2, b2), rview(GH3, R2, m, a2, b2)
    be.stt(scr(5), G3L[:, 1::2, :], -rg, G3L[:, 0::2, :])
    be.stt(scr(6), G3H[:, 1::2, :], -rg, G3H[:, 0::2, :])

    if rows is None or b == R:
        s_band = s * (f0 * f0) * (g0 * g0)
        be.scale_copy(dst_tile[:, dst_off:dst_off + 7 * q], SCR[:, :], s_band)
    return LLn, s * f0 * f0


def emit_chunk(be, t, q, LLD, nb, start, nq=4):
    """Load chunk (t, q of nq) and reduce to its LLd rows."""
    rw = 64 // nq                  # x-rows per partition in this chunk
    XT = be.alloc("xt", [128, rw * 512])
    be.load_chunk(t, q, XT, nq)
    X3 = be.view(XT, (rw, 512))
    U = be.alloc("u", [128, (rw // 2) * 512])
    be.add(be.view(U, (rw // 2, 512)), X3[:, 0::2, :], X3[:, 1::2, :])
    U3 = be.view(U, (rw // 2, 512))
    L0C = be.alloc("l0c", [128, (rw // 2) * 256])
    be.add(be.view(L0C, (rw // 2, 256)), U3[:, :, 0::2], U3[:, :, 1::2])
    L3 = be.view(L0C, (rw // 2, 256))
    U2 = be.alloc("u2", [128, (rw // 4) * 256])
    be.add(be.view(U2, (rw // 4, 256)), L3[:, 0::2, :], L3[:, 1::2, :])
    U23 = be.view(U2, (rw // 4, 256))
    lld4 = be.view(LLD, (16, nb, 128))
    nr = rw // 4                   # LLd rows produced by this chunk
    be.add(lld4[:, nr * q:nr * q + nr, t - start, :],
           U23[:, :, 0::2], U23[:, :, 1::2])


def emit_core(be, h, g):
    c = INV_SQRT2
    opts = getattr(be, 'opts', {})
    sets = opts.get('sets', list(DEF_SETS))
    split_last = opts.get('split_l0', False)
    offs = region_offsets(sets)
    TAILLL = be.alloc("tailll", [PPC, 256])

    LLDs, OUTBs, bases, lens = {}, {}, {}, {}
    for start, nb in sets:
        LLDs[start] = be.alloc(f"lld{start}", [128, 16 * nb * 128])
        base = offs[(start, 0)][0]
        ln = sum(offs[(start, l)][1] for l in range(3))
        bases[start], lens[start] = base, ln
        OUTBs[start] = be.alloc(f"outb{start}", [128, ln], dtype='f32')

    last_nq = opts.get('last_nq', 8)
    s_dir = None
    # chunks in (t, q) order; inject each set's directions right after the
    # last chunk of its batches so they overlap the remaining loads
    for si, (start, nb) in enumerate(sets):
        LLD = LLDs[start]
        OUTB_s, base = OUTBs[start], bases[start]
        loff = [offs[(start, l)][0] - base for l in range(3)]
        is_last = si == len(sets) - 1
        nq = last_nq if is_last else 4
        split = split_last and is_last and nb == 1 and nq == 4
        if split:
            # l=0 in row-halves interleaved with the chunk loads, so the
            # first half hides under the last loads
            t = start
            emit_chunk(be, t, 0, LLD, nb, start)
            emit_chunk(be, t, 1, LLD, nb, start)
            tiles = {}
            s0 = c ** 4
            emit_direction(be, LLD, 16, 128, nb, 0, s0, OUTB_s, loff[0],
                           128, h, g, rows=(0, 8), tiles=tiles)
            emit_chunk(be, t, 2, LLD, nb, start)
            emit_chunk(be, t, 3, LLD, nb, start)
            LL, s = emit_direction(be, LLD, 16, 128, nb, 0, s0, OUTB_s,
                                   loff[0], 128, h, g, rows=(8, 16),
                                   tiles=tiles)
            R, S = 8, 64
            for l in (1, 2):
                LL, s = emit_direction(be, LL, R, S, nb, l, s, OUTB_s,
                                       loff[l], 128, h, g)
                R, S = R // 2, S // 2
        else:
            for t in range(start, start + nb):
                for q in range(nq):
                    emit_chunk(be, t, q, LLD, nb, start, nq)
            if si == 0:
                # tail-matmul weights + identity: issued here so their
                # gpsimd work doesn't delay the first loads' descriptor gen
                be.prep_tail_mm()
            LL, s, R, S = LLD, c ** 4, 16, 128
            for l in range(3):
                LL, s = emit_direction(be, LL, R, S, nb, l, s, OUTB_s,
                                       loff[l], 128, h, g)
                R, S = R // 2, S // 2
        s_dir = s
        be.repack_set(LL, start, nb, TAILLL)
        if opts.get('early_stores', False) and not is_last:
            be.store_outb_set(OUTB_s, bases[start], lens[start])

    # remaining output stores at the end: mid-stream stores can contend
    # with the input loads; parallel small DMAs finish in ~4us
    for si, (start, nb) in enumerate(sets):
        if opts.get('early_stores', False) and si != len(sets) - 1:
            continue
        be.store_outb_set(OUTBs[start], bases[start], lens[start])

    # ---- tail: all 273 remaining outputs as one matmul ---------------------
    be.emit_tail_mm(TAILLL)
    return s_dir


def emit_tail_dve(be, TAILLL, TMAP, OUTT, s, h, g):
    """DVE tail (mirror/probe only): directions l=3,4,5 + rank-1 consts."""
    LL, R, S = TAILLL, 16, 16
    for l in (3, 4, 5):
        LL, s = emit_direction(be, LL, R, S, 1, l, s, OUTT, TOFF[l],
                               PPC, h, g)
        R, S = R // 2, S // 2
    CT = be.alloc("ct", [PPC, 2])
    L3 = be.view(LL, (2, 2))
    be.add(be.view(CT, (2, 1)), L3[:, :, 0:1], L3[:, :, 1:2])
    LL11 = be.alloc("ll11", [PPC, 1], dtype='f32')
    be.add(LL11[:, 0:1], CT[:, 0:1], CT[:, 1:2])
    be.ts_mul(OUTT[:, TCONST:TCONST + 126], TMAP[:, :], LL11[:, 0:1])


# ---- host-side constants --------------------------------------------------
def _dwt2_np(x, f0, f1):
    def dwt_last(x):
        n = x.shape[-1]
        m = (n + 1) // 2
        xe = np.pad(x, [(0, 0)] * (x.ndim - 1) + [(1, 1)], mode='edge')
        a = xe[..., 1:2 * m + 1:2]
        b = xe[..., 2:2 * m + 2:2]
        return f1 * a + f0 * b, f0 * a - f1 * b

    lo, hi = dwt_last(x)
    lo, hi = np.swapaxes(lo, -1, -2), np.swapaxes(hi, -1, -2)
    ll, lh = dwt_last(lo)
    hl, hh = dwt_last(hi)
    sw = lambda t: np.swapaxes(t, -1, -2)
    return sw(ll), sw(lh), sw(hl), sw(hh)


def build_tail_consts(h, g, s11):
    c = INV_SQRT2
    h = np.asarray(h, np.float64)
    g = np.asarray(g, np.float64)
    LL = np.ones((1, 1))
    vals = []
    for k in range(1, 4):
        if k > 1:
            LL, _, _, _ = _dwt2_np(LL, c, c)
        for l in range(6):
            LL, LH, HL, HH = _dwt2_np(LL, h[l, 0], h[l, 1])
            A1, H1, V1, _ = _dwt2_np(LH, g[l, 0], g[l, 1])
            A2, H2, _, _ = _dwt2_np(HL, g[l, 0], g[l, 1])
            _, H3, _, D3 = _dwt2_np(HH, g[l, 0], g[l, 1])
            for sb in (A1, H1, V1, A2, H2, H3, D3):
                vals.append(float(sb[0, 0]))
    return (np.asarray(vals, np.float64) * s11).astype(np.float32)


def tail_scale(h, g):
    c = INV_SQRT2
    s = c ** 4
    for l in range(6):
        s *= float(h[l, 0]) ** 2
    return s * c * c


def build_mtail(h, g):
    """[256, 273] f32: the full tail (l=3,4,5 bands + scale 2-4 consts) as a
    linear map of the raw (dropped-scale) 16x16 TAILLL values, probed by
    running the DVE-tail mirror on basis vectors, 64 per run."""
    c = INV_SQRT2
    s3 = c ** 4
    for l in range(3):
        s3 *= float(h[l, 0]) ** 2
    tmap_row = build_tail_consts(h, g, tail_scale(h, g))
    M = np.zeros((256, 273), np.float32)
    for grp in range(4):
        be = NumpyBE(None, h, g, tmap_row)
        TAILLL = NpTile(np.zeros((PPC, 256), np.float32))
        for p in range(PPC):
            TAILLL.arr[p, grp * PPC + p] = 1.0
        TMAP = be.alloc("tmap", [PPC, 126])
        be.load_tmap(TMAP)
        OUTT = be.alloc("outt", [PPC, TAIL_FLOATS])
        emit_tail_dve(be, TAILLL, TMAP, OUTT, s3, h, g)
        M[grp * PPC:(grp + 1) * PPC] = OUTT.arr
    return M


def build_perm2():
    """M[b, ref_pos] for the scale-1 part: index into the per-pl flattened
    [8 blk, 4704] array; plus tail permutation (identity layout).
    Device OUT_BLK partition = (pl, blk); region l = [sb7, r(R4), b4, m]."""
    offs = region_offsets(DEF_SETS)
    M = np.empty((NBATCH, 9408), np.int64)
    for start, nb in DEF_SETS:
        for bl in range(nb):
            b = start + bl
            roff = 0
            for l, mm in enumerate((32, 16, 8)):
                R4 = mm // NBLK        # band rows per partition (4, 2, 1)
                loff = offs[(start, l)][0]
                for sb in range(7):
                    for row in range(mm):
                        blk, rl = divmod(row, R4)
                        base = (blk * BLK_FLOATS + loff
                                + ((sb * R4 + rl) * nb + bl) * mm)
                        M[b, roff + sb * mm * mm + row * mm:
                          roff + sb * mm * mm + (row + 1) * mm] = \
                            np.arange(base, base + mm)
                roff += 7 * mm * mm
            assert roff == 9408
    return M


def gather_host(out_blk, out_tail, M):
    """out_blk [128, 4704], out_tail [64, 273] -> [64, 9681] ref order."""
    v = out_blk.reshape(BPL, NBLK * BLK_FLOATS)
    out = np.empty((PPC, 9681), np.float32)
    for b in range(NBATCH):
        out[b * BPL:(b + 1) * BPL, :9408] = v[:, M[b]]
    out[:, 9408:] = out_tail
    return out


# ---- device backend -------------------------------------------------------
class BassBE:
    def __init__(self, tc, pools, xs_ap, mtail_ap, outblk_ap, outtail_ap,
                 bounce_ap, opts=None):
        self.opts = opts or {}
        self.tc = tc
        self.nc = tc.nc
        self.pools = pools
        self.xs = xs_ap
        self.mtail_dram = mtail_ap
        self.outblk = outblk_ap
        self.outtail = outtail_ap
        self.bounce = bounce_ap

    def alloc(self, name, shape, dtype='bf16'):
        from concourse import mybir
        dt = mybir.dt.float32 if dtype == 'f32' else mybir.dt.bfloat16
        if (name in ('tailll', 'tmap', 'outt') or name.startswith('lld')
                or name.startswith('outb')):
            pool = self.pools['persist']
        elif name == 'xt':
            pool = self.pools['xt']
        else:
            pool = self.pools['work']
        return pool.tile(list(shape), dt, tag=name, name=name)

    @staticmethod
    def view(tile, dims, sub=None):
        ap = tile[:, :]
        if sub is not None:
            ap = ap[:, sub[0]:sub[0] + sub[1]]
        if len(dims) == 2:
            return ap.rearrange("p (r c) -> p r c", c=dims[1])
        P, F = ap.shape
        assert F == dims[0] * dims[1] * dims[2]
        return ap.rearrange("p (r b c) -> p r b c", b=dims[1], c=dims[2])

    def add(self, out, a, b):
        self.nc.vector.tensor_add(out=out, in0=a, in1=b)

    def stt(self, out, a, s, b):
        from concourse import mybir
        self.nc.vector.scalar_tensor_tensor(
            out=out, in0=a, scalar=float(s), in1=b,
            op0=mybir.AluOpType.mult, op1=mybir.AluOpType.add)

    def scale_copy(self, out, inp, s):
        self.nc.scalar.mul(out, inp, float(s))

    def ts_mul(self, out, a, col):
        from concourse import mybir
        self.nc.vector.tensor_scalar(
            out=out, in0=a, scalar1=col, scalar2=None,
            op0=mybir.AluOpType.mult)

    def load_chunk(self, t, q, dst, nq=4):
        v = self.xs.rearrange("pl (blk q r) c -> pl blk q r c", blk=NBLK, q=nq)
        v = v[t * BPL:(t + 1) * BPL, :, q]
        v = v.rearrange("pl blk r c -> (pl blk) (r c)")
        self.nc.gpsimd.dma_start(out=dst[:, :], in_=v)   # f32 -> bf16 cast

    def repack_set(self, ll3, start, nb, tail):
        # per-batch SBUF->SBUF: [128=(pl,blk), (r2,c16) at b] ->
        # tail rows [pl, (blk r c)]; falls back via DRAM if the AP layer
        # rejects the partition-split source
        v = ll3[:, :].rearrange("p (r b c) -> p r b c", b=nb, c=16)
        for i in range(nb):
            b = start + i
            try:
                src = v[:, :, i, :].rearrange(
                    "(pl blk) r c -> pl (blk r c)", blk=NBLK)
                self.nc.sync.dma_start(
                    out=tail[b * BPL:(b + 1) * BPL, :], in_=src)
            except Exception:
                dst = self.bounce[b].rearrange(
                    "pl (blk r c) -> (pl blk) (r c)", blk=NBLK, r=2)
                self.nc.sync.dma_start(out=dst, in_=v[:, :, i, :])
                self.nc.sync.dma_start(out=tail[b * BPL:(b + 1) * BPL, :],
                                       in_=self.bounce[b])

    def store_outb_set(self, outb, base, ln):
        self.nc.sync.dma_start(out=self.outblk[:, base:base + ln],
                               in_=outb[:, :])

    def prep_tail_mm(self):
        from concourse import mybir
        from concourse.masks import make_identity
        p = self.pools['persist']
        self.MT0 = p.tile([128, 273], mybir.dt.bfloat16, tag='mt0', name='mt0')
        self.MT1 = p.tile([128, 273], mybir.dt.bfloat16, tag='mt1', name='mt1')
        self.nc.gpsimd.dma_start(out=self.MT0[:, :], in_=self.mtail_dram[0:128])
        self.nc.gpsimd.dma_start(out=self.MT1[:, :],
                                 in_=self.mtail_dram[128:256])
        self.IDENT = p.tile([64, 64], mybir.dt.bfloat16, tag='id', name='id')
        make_identity(self.nc, self.IDENT[:, :])

    def emit_tail_mm(self, tail):
        from concourse import mybir
        nc = self.nc
        psum = self.pools['psum']
        wp = self.pools['work']
        T0p = psum.tile([128, 64], mybir.dt.bfloat16, tag='t0p', name='t0p')
        T1p = psum.tile([128, 64], mybir.dt.bfloat16, tag='t1p', name='t1p')
        nc.tensor.transpose(T0p[:, :], tail[:, 0:128], self.IDENT[:, :])
        nc.tensor.transpose(T1p[:, :], tail[:, 128:256], self.IDENT[:, :])
        T0s = wp.tile([128, 64], mybir.dt.bfloat16, tag='t0s', name='t0s')
        T1s = wp.tile([128, 64], mybir.dt.bfloat16, tag='t1s', name='t1s')
        nc.vector.tensor_copy(out=T0s[:, :], in_=T0p[:, :])
        nc.vector.tensor_copy(out=T1s[:, :], in_=T1p[:, :])
        OT = wp.tile([128, 3 * 64], mybir.dt.float32, tag='ot', name='ot')
        for mc, (m0, m1) in enumerate(((0, 128), (128, 256), (256, 273))):
            W = m1 - m0
            P0 = psum.tile([128, 64], mybir.dt.float32,
                           tag=f'pmm{mc}', name=f'pmm{mc}')
            nc.tensor.matmul(P0[0:W, :], self.MT0[:, m0:m1], T0s[:, :],
                             start=True, stop=False)
            nc.tensor.matmul(P0[0:W, :], self.MT1[:, m0:m1], T1s[:, :],
                             start=False, stop=True)
            nc.scalar.copy(OT[0:W, mc * 64:(mc + 1) * 64], P0[0:W, :])
        outT = self.outtail.rearrange("mc p n -> p mc n")
        src = OT[:, :].rearrange("p (mc n) -> p mc n", n=64)
        nc.sync.dma_start(out=outT, in_=src)


def build_program(h, g, opts=None):
    from contextlib import ExitStack
    import concourse.bacc as bacc
    import concourse.tile as tile
    from concourse import mybir

    from concourse.bass import MemorySpace

    opts = opts or {}
    nc = bacc.Bacc("TRN2", target_bir_lowering=False, debug=False,
                   num_devices=NCORES)
    xs = nc.dram_tensor("xs", [PPC, 512, 512], mybir.dt.float32,
                        kind="ExternalInput").ap()
    mtail = nc.dram_tensor("mtail", [256, 273], mybir.dt.float32,
                           kind="ExternalInput").ap()
    outblk = nc.dram_tensor("out_blk", [128, BLK_FLOATS],
                            mybir.dt.float32, kind="ExternalOutput").ap()
    outtail = nc.dram_tensor("out_tail", [3, 128, 64],
                             mybir.dt.float32, kind="ExternalOutput").ap()
    bounce = nc.dram_tensor("bounce", [NBATCH, BPL, 256],
                            mybir.dt.bfloat16).ap()

    with ExitStack() as ctx:
        tc = ctx.enter_context(tile.TileContext(nc, trace_sim=False))
        pools = {
            'xt': ctx.enter_context(
                tc.tile_pool(name="xt", bufs=opts.get('xt_bufs', 5))),
            'work': ctx.enter_context(
                tc.tile_pool(name="work", bufs=opts.get('work_bufs', 1))),
            'persist': ctx.enter_context(tc.tile_pool(name="persist", bufs=1)),
            'psum': ctx.enter_context(
                tc.tile_pool(name="psum", bufs=1, space=MemorySpace.PSUM)),
        }
        be = BassBE(tc, pools, xs, mtail, outblk, outtail, bounce, opts)
        emit_core(be, h, g)
    nc.compile()
    return nc


# ---- public entry ---------------------------------------------------------
_CACHE = {}
LAST_EXEC_NS = None


def kernel(x, h, g):
    x = np.ascontiguousarray(np.asarray(x), dtype=np.float32)
    h = np.asarray(h, np.float32)
    g = np.asarray(g, np.float32)
    B, C = x.shape[0], x.shape[1]

    from concourse.bass_utils import run_bass_kernel_spmd

    key = (h.tobytes(), g.tobytes())
    if key not in _CACHE:
        nc = build_program(h, g)
        mtail = np.ascontiguousarray(build_mtail(h, g), dtype=np.float32)
        M = build_perm2()
        _CACHE[key] = (nc, mtail, M)
    nc, mtail, M = _CACHE[key]

    planes = x.reshape(NPLANES, 512, 512)
    in_maps = [{"xs": planes[k * PPC:(k + 1) * PPC], "mtail": mtail}
               for k in range(NCORES)]
    res = run_bass_kernel_spmd(nc, in_maps, list(range(NCORES)))
    global LAST_EXEC_NS
    LAST_EXEC_NS = getattr(res, 'exec_time_ns', None)
    out = np.empty((NPLANES, 9681), np.float32)
    for k in range(NCORES):
        tail = res.results[k]["out_tail"].reshape(384, 64)[:273].T
        out[k * PPC:(k + 1) * PPC] = gather_host(
            res.results[k]["out_blk"], tail, M)
    return out.reshape(B, C, 9681)


# ---- numpy-mirror full run (for testing without HW) -----------------------
def kernel_numpy(x, h, g):
    x = np.asarray(x, np.float32)
    h = np.asarray(h, np.float32)
    g = np.asarray(g, np.float32)
    B, C = x.shape[0], x.shape[1]
    tmap_row = build_tail_consts(h, g, tail_scale(h, g))
    mtail = build_mtail(h, g)
    M = build_perm2()
    planes = x.reshape(NPLANES, 512, 512)
    out = np.empty((NPLANES, 9681), np.float32)
    for k in range(NCORES):
        be = NumpyBE(planes[k * PPC:(k + 1) * PPC], h, g, tmap_row, mtail)
        emit_core(be, h, g)
        out[k * PPC:(k + 1) * PPC] = gather_host(be.out_blk, be.out_tail, M)
    return out.reshape(B, C, 9681)
